# revision 18
# baseline (speedup 1.0000x reference)
"""GCN (2-layer) + edge-dot decode on 8 TRN2 NeuronCores.

Math (per GCN layer, with dinv = rsqrt(indeg+1)):
    out[v] = dinv[v] * ( sum_{e: dst=v} hs[src_e] + hs[v] ) + b,  hs = dinv (.) (x @ W)
so no per-edge norm values are needed anywhere; all scaling is per-node.

Layer 2 is commuted past W2:  z = (dinv (.) (A_hat g)) @ W2 + b2,  g = dinv (.) relu(out1).

Device layout: nodes permuted (degree-sorted, core-striped so each core owns a
contiguous 6272-row slice). Aggregation via dma_gather row-gathers + TensorE
identity-matmul accumulation into PSUM, 4 rounds packed per 512-wide bf16
matmul (the 4 PSUM column groups are summed by one strided DVE reduce).
ALL tables are bf16 (RDH AllGather runs ~4x faster on bf16 than fp32): hs and
g tables [NP, 128] bf16, z table [NP, 128] bf16 with z in cols 0:64 and zero
pad in 64:128 (gather elem must be >=256B). Tables AllGather in 7 sub-chunks
with Shared outputs, fired per 7-block group; aggregation phases process
blocks in REVERSE degree order so the slowest blocks (and the last sub-AG)
land at the phase end with the collective queue already drained. Gathers
round-robin over 4 SWDGE queues; trailing-pad runs are 16 idxs (not a full
128 round).

dma_gather indices are SIGNED int16, so every gather uses a frame centered at
row 32768 (idx = row - 32768 covers the whole 50176-row table); the ucode
drops a trailing run of negative indices, so each gather appends 16 positive
pad idxs pointing at a zero (pad-node) row.
"""

import sys
import numpy as np
from contextlib import ExitStack

sys.path.insert(0, "/opt/trn_rl_repo")

import ml_dtypes
import concourse.bass as bass
import concourse.mybir as mybir
from concourse.bass_utils import run_bass_kernel_spmd
from concourse.tile import TileContext, add_dep_helper
from concourse.masks import make_identity
from concourse.library_config import mlp
from concourse.library_overlay import lower_extended_insts

N, E, L = 50000, 800000, 200000
IN, HID, OUT = 256, 128, 64
C = 8                      # cores
NP = 50176                 # padded node count = 392 blocks of 128
NPC = NP // C              # 6272 nodes per core
BPC = NPC // 128           # 49 blocks per core
FBASE = 32768              # gather frame base row (signed int16 centered)
PADIDX = NP - 1 - FBASE    # pad index -> row 50175 (a zero pad-node row), > 0
CH_MAX = 24                # max rounds per gather chunk (excl. appended pad idxs)
DEC_CHUNK = 16             # decode chunk rounds
BOUND = [0, 7, 14, 21, 28, 35, 42, 49]       # AG sub-chunk block boundaries
NSUB = len(BOUND) - 1      # AllGather sub-chunks per table
GQ = 4                     # SWDGE gather queues
G1 = 4                     # rounds packed per matmul (4*128 = 512)
KMIN = 8                   # min rounds per block (ensures a full first pack)
NPAD = 16                  # trailing positive pad idxs per gather

BF16 = mybir.dt.bfloat16

CUSTOM_ISA_OPCODES = {"DMAGatherAnt", "DMAScatterAddAnt"}


def _fix_sync_waits(nc):
    """This container's walrus accepts at most one sync-wait per instruction
    and none on custom ISA ucode ops; hoist extras onto preceding drains."""
    f = nc.m.functions[0]
    for b in f.blocks:
        insts = b.instructions
        i = 0
        while i < len(insts):
            ins = insts[i]
            si = ins.sync_info
            nw = len(si.on_wait) if (si is not None and si.on_wait is not None) else 0
            keep = 0 if str(ins.opcode) in CUSTOM_ISA_OPCODES else 1
            if nw > keep:
                waits = list(si.on_wait)
                hoist, keepw = waits[: nw - keep], waits[nw - keep:]
                for j, w in enumerate(hoist):
                    d = mybir.InstEventSemaphore(name=f"{ins.name}-wsplit{j}")
                    d.engine = ins.engine
                    d.sync_info = mybir.SyncInfo(on_wait=[w], on_update=[])
                    insts.insert(i + j, d)
                si.on_wait = keepw
                i += len(hoist)
            i += 1


def _sortedpos(p):
    """final position -> position in the degree-sorted sequence."""
    core = p // NPC
    k = (p % NPC) // 128
    lane = p % 128
    return 128 * (8 * k + core) + lane


def _rowmap(p):
    """final position -> gather-table row: [all cores' sub 0][all cores' sub 1]...
    so each own sub-chunk AllGathers to a contiguous table span."""
    c = p // NPC
    o = p % NPC
    b = o // 128
    gi = np.searchsorted(np.asarray(BOUND), b, side="right") - 1
    lo, hi = np.asarray(BOUND)[gi] * 128, np.asarray(BOUND)[np.minimum(gi + 1, NSUB)] * 128
    return (lo * C + c * (hi - lo)) + (o - lo)


def _wrap_idx(flat):
    """[n] int16 -> [128, n//16] wrapped in 16 partitions, replicated x8."""
    n = flat.shape[0]
    arr = np.empty((16, n // 16), dtype=np.int16)
    arr[:, :] = flat.reshape(n // 16, 16).T
    return np.tile(arr, (8, 1))


def _prepare(edge_index, edge_label_index):
    src = np.asarray(edge_index[0], dtype=np.int64)
    dst = np.asarray(edge_index[1], dtype=np.int64)
    la = np.asarray(edge_label_index[0], dtype=np.int64)
    lb = np.asarray(edge_label_index[1], dtype=np.int64)

    deg = np.bincount(dst, minlength=N).astype(np.int64)
    assert np.array_equal(np.sort(_rowmap(np.arange(NP))), np.arange(NP))
    assert _rowmap(np.asarray(NP - 1)) == NP - 1

    # permutation: degree-sorted, core-striped; 176 zero pad nodes at the tail
    sorted_real = np.argsort(-deg, kind="stable")
    seq = np.full(NP, -1, dtype=np.int64)
    seq[:N] = sorted_real
    final_perm = seq[_sortedpos(np.arange(NP))]   # final position -> orig (-1 pad)
    real_mask = final_perm >= 0
    invpos = np.full(N, -1, dtype=np.int64)
    invpos[final_perm[real_mask]] = np.nonzero(real_mask)[0]
    assert final_perm[NP - 1] == -1

    ps = invpos[src]
    pd = invpos[dst]

    # per-node in-edge ranks (dst-major)
    order = np.argsort(pd, kind="stable")
    pd_s = pd[order]
    ps_s = ps[order]
    newgrp = np.empty(E, dtype=bool)
    newgrp[0] = True
    newgrp[1:] = pd_s[1:] != pd_s[:-1]
    gidx = np.nonzero(newgrp)[0]
    rank = np.arange(E) - gidx[np.cumsum(newgrp) - 1]

    lane = pd_s % 128
    core = pd_s // NPC
    slot = (pd_s % NPC) // 128

    nblocks = NP // 128
    KB = np.zeros(nblocks, dtype=np.int64)
    np.maximum.at(KB, pd_s // 128, rank + 1)
    Khat = np.zeros(BPC, dtype=np.int64)
    for k in range(BPC):
        Khat[k] = max(int(KB[[c * BPC + k for c in range(C)]].max()), KMIN)
    off = np.zeros(BPC + 1, dtype=np.int64)
    off[1:] = np.cumsum(Khat)

    idxT = np.full((C, off[-1], 128), PADIDX, dtype=np.int16)
    idxT[core, off[slot] + rank, lane] = (_rowmap(ps_s) - FBASE).astype(np.int16)

    # chunk schedule per block: rounds split into chunks of <= CH_MAX, each
    # gather = chunk rounds + NPAD appended pad idxs (trailing positivity)
    chunks = []   # (k, r0, ch)
    for k in range(BPC):
        r = 0
        while r < int(Khat[k]):
            ch = min(CH_MAX, int(Khat[k]) - r)
            chunks.append((k, r, ch))
            r += ch

    # degp per core [128, BPC]
    degp = np.full(NP, 1e30, dtype=np.float32)
    degp[real_mask] = (deg[final_perm[real_mask]] + 1).astype(np.float32)
    degp_core = degp.reshape(C, BPC, 128).transpose(0, 2, 1).copy()

    # decode tables: natural label order per core, chunks of DEC_CHUNK rounds
    pa = invpos[la]
    pb = invpos[lb]
    LPC = L // C
    LROUNDS = (LPC + 127) // 128
    dec_chunks = []
    r = 0
    while r < LROUNDS:
        ch = min(DEC_CHUNK, LROUNDS - r)
        dec_chunks.append((r, ch))
        r += ch

    # label reorder: labels whose BOTH endpoint table rows avoid the
    # LAST-FIRED AG3 group (blocks BOUND[0]:BOUND[1], fired last since phases
    # process groups in reverse) go first -- their decode gathers only need
    # the first NSUB-1 sub-AGs and can start during the AG3 tail.
    brows = np.asarray([b * C * 128 for b in BOUND])

    def grp_of(row):
        return np.searchsorted(brows, row, side="right") - 1

    ra = _rowmap(pa)
    rb = _rowmap(pb)
    gmin = np.minimum(grp_of(ra), grp_of(rb))   # earliest-needed AG3 group
    # class 2: both endpoints avoid the last TWO fired groups (0 and 1);
    # class 1: avoid the last-fired group (0); class 0: the rest.
    klass = np.where(gmin >= 2, 2, np.where(gmin >= 1, 1, 0)).astype(np.int64)
    label_order = np.empty((C, LPC), dtype=np.int64)
    ne2, ne1 = [], []
    for c in range(C):
        kc = klass[c * LPC:(c + 1) * LPC]
        label_order[c] = np.argsort(-kc, kind="stable")
        ne2.append(int((kc == 2).sum()))
        ne1.append(int((kc >= 1).sum()))
    n_e2_rounds = min(ne2) // 128             # rounds decodable after cc3s[:-2]
    n_early_rounds = min(ne1) // 128          # rounds decodable after cc3s[:-1]

    pad16 = np.full(NPAD, PADIDX, dtype=np.int16)
    idx16 = []
    for c in range(C):
        parts = []
        for (k, r0, ch) in chunks:
            flat = idxT[c, off[k] + r0: off[k] + r0 + ch].reshape(-1)
            parts.append(_wrap_idx(np.concatenate([flat, pad16])))
        pa_c = pa[c * LPC:(c + 1) * LPC][label_order[c]]
        pb_c = pb[c * LPC:(c + 1) * LPC][label_order[c]]
        fa = np.full(LROUNDS * 128, PADIDX, dtype=np.int16)
        fb = np.full(LROUNDS * 128, PADIDX, dtype=np.int16)
        fa[:LPC] = (_rowmap(pa_c) - FBASE).astype(np.int16)
        fb[:LPC] = (_rowmap(pb_c) - FBASE).astype(np.int16)
        for (r0, ch) in dec_chunks:
            parts.append(_wrap_idx(np.concatenate([fa[r0 * 128:(r0 + ch) * 128],
                                                   fb[r0 * 128:(r0 + ch) * 128], pad16])))
        idx16.append(np.ascontiguousarray(np.concatenate(parts, axis=1)))

    return dict(
        final_perm=final_perm, invpos=invpos, real_mask=real_mask,
        Khat=Khat, off=off, chunks=chunks, dec_chunks=dec_chunks,
        degp_core=degp_core, idx16=idx16, label_order=label_order,
        n_early_rounds=n_early_rounds, n_e2_rounds=n_e2_rounds,
    )


def _cwidth(ch):
    return ch * 8 + NPAD // 16        # idx cols per gather chunk


def _nidx(ch):
    return ch * 128 + NPAD            # gather idx count per chunk


def _build(prep):
    chunks = prep["chunks"]
    dec_chunks = prep["dec_chunks"]
    TOTW = prep["idx16"][0].shape[1]
    ndec_cols = sum(ch for (_, ch) in dec_chunks)

    nc = bass.Bass(num_devices=C, dynamic_dma_scratch_size=32768, num_swdge_queues=GQ)
    f32 = mybir.dt.float32
    xT_d = nc.dram_tensor("xT", [IN, NPC], BF16, kind="ExternalInput")
    W1_d = nc.dram_tensor("W1", [IN, HID], BF16, kind="ExternalInput")
    b1_d = nc.dram_tensor("b1", [1, HID], f32, kind="ExternalInput")
    W2_d = nc.dram_tensor("W2", [HID, OUT], BF16, kind="ExternalInput")
    b2_d = nc.dram_tensor("b2", [1, OUT], f32, kind="ExternalInput")
    degp_d = nc.dram_tensor("degp", [128, BPC], f32, kind="ExternalInput")
    idx_d = nc.dram_tensor("idx16", [128, TOTW], mybir.dt.int16, kind="ExternalInput")
    out_d = nc.dram_tensor("out", [128, ndec_cols], f32, kind="ExternalOutput")

    ag1_in = nc.dram_tensor("ag1_in", [NPC, HID], BF16)
    tab1 = nc.dram_tensor("tab1", [NP, HID], BF16, addr_space="Shared")
    ag2_in = nc.dram_tensor("ag2_in", [NPC, HID], BF16)
    tab2 = nc.dram_tensor("tab2", [NP, HID], BF16, addr_space="Shared")
    ag3_in = nc.dram_tensor("ag3_in", [NPC, 128], BF16)
    ztab = nc.dram_tensor("ztab", [NP, 128], BF16, addr_space="Shared")

    with TileContext(nc) as tc, ExitStack() as ctx:
        const = ctx.enter_context(tc.tile_pool(name="const", bufs=1))
        own = ctx.enter_context(tc.tile_pool(name="own", bufs=1))
        lp = ctx.enter_context(tc.tile_pool(name="lhsT", bufs=6))
        gp = ctx.enter_context(tc.tile_pool(name="gath", bufs=6))
        dgp = ctx.enter_context(tc.tile_pool(name="dgath", bufs=4))
        pp = ctx.enter_context(tc.tile_pool(name="psA", bufs=4, space="PSUM"))
        pzg = ctx.enter_context(tc.tile_pool(name="psG", bufs=2, space="PSUM"))
        pzt = ctx.enter_context(tc.tile_pool(name="psT", bufs=1, space="PSUM"))
        pzy = ctx.enter_context(tc.tile_pool(name="psY", bufs=1, space="PSUM"))
        sp_ = ctx.enter_context(tc.tile_pool(name="stage", bufs=4))
        prp = ctx.enter_context(tc.tile_pool(name="prod", bufs=2))

        ll = nc.gpsimd.load_library(mlp)

        ident = const.tile([128, 128], f32)
        make_identity(nc, ident[:])
        ident_bf = const.tile([128, 128], BF16)
        nc.scalar.activation(out=ident_bf[:], in_=ident[:],
                             func=mybir.ActivationFunctionType.Copy)

        idx_sb = const.tile([128, TOTW], mybir.dt.int16)
        idma = nc.sync.dma_start(out=idx_sb[:], in_=idx_d[:, :])
        add_dep_helper(idma.ins, ll.ins, reason="idx after lib load")

        kvals = sorted({_nidx(ch) for (_, _, ch) in chunks}
                       | {_nidx(2 * ch) for (_, ch) in dec_chunks})
        kreg = {}
        for v in kvals:
            r = ctx.enter_context(nc.gpsimd.register(f"nidx{v}"))
            nc.gpsimd.reg_mov(r, v)
            kreg[v] = r

        W1_sb = []
        for i in range(2):
            w1t = const.tile([128, HID], BF16, tag=f"w1_{i}", name=f"w1_{i}")
            nc.sync.dma_start(out=w1t[:], in_=W1_d[i * 128:(i + 1) * 128, :])
            W1_sb.append(w1t)
        W2_sb = const.tile([128, OUT], BF16)
        nc.sync.dma_start(out=W2_sb[:], in_=W2_d[:, :])

        ones_row = const.tile([1, 128], f32)
        nc.vector.memset(ones_row[:], 1.0)
        b1_row = const.tile([1, HID], f32)
        nc.sync.dma_start(out=b1_row[:], in_=b1_d[:, :])
        b2_row = const.tile([1, OUT], f32)
        nc.sync.dma_start(out=b2_row[:], in_=b2_d[:, :])
        bias1 = const.tile([128, HID], f32)
        bps = pzg.tile([128, HID], f32, tag="g1")
        nc.tensor.matmul(out=bps[:], lhsT=ones_row[:], rhs=b1_row[:], start=True, stop=True)
        nc.scalar.activation(out=bias1[:], in_=bps[:], func=mybir.ActivationFunctionType.Copy)
        bias2 = const.tile([128, OUT], f32)
        bps2 = pzy.tile([128, OUT], f32, tag="y")
        nc.tensor.matmul(out=bps2[:], lhsT=ones_row[:], rhs=b2_row[:], start=True, stop=True)
        nc.scalar.activation(out=bias2[:], in_=bps2[:], func=mybir.ActivationFunctionType.Copy)

        degp_sb = const.tile([128, BPC], f32)
        nc.sync.dma_start(out=degp_sb[:], in_=degp_d[:, :])
        rec = const.tile([128, BPC], f32)
        nc.vector.reciprocal(out=rec[:], in_=degp_sb[:])
        dinv = const.tile([128, BPC], f32)
        nc.scalar.activation(out=dinv[:], in_=rec[:], func=mybir.ActivationFunctionType.Sqrt)

        hs_own = own.tile([128, NPC], BF16)
        g_own = own.tile([128, NPC], BF16)

        # chunk -> idx column offsets
        blk_chunks = [[] for _ in range(BPC)]
        co = 0
        for (k, r0, ch) in chunks:
            blk_chunks[k].append((co, ch))
            co += _cwidth(ch)
        dec_offs = []
        for (r0, ch) in dec_chunks:
            dec_offs.append(co)
            co += _cwidth(2 * ch)
        assert co == TOTW

        qctr = [0]

        def next_q(n):
            q = qctr[0]
            qctr[0] = (qctr[0] + 1) % GQ
            return q

        def sub_allgather(ag_in, tab, blo, bhi, wrs, scope):
            """AllGather own rows [blo*128, bhi*128) -> table rows
            [blo*C*128, bhi*C*128). Contiguous by _rowmap for any union of
            adjacent BOUND groups."""
            lo, hi = blo * 128, bhi * 128
            with nc.named_scope(scope):
                cc = nc.gpsimd.collective_compute(
                    "AllGather", mybir.AluOpType.bypass,
                    replica_groups=[list(range(C))],
                    ins=[ag_in[lo:hi, :].opt()],
                    outs=[tab[lo * C:hi * C, :].opt()],
                )
                for w in wrs:
                    add_dep_helper(cc.ins, w.ins, reason=f"{scope} in-rows")
            return cc

        # ---------------- Phase A: GEMM1 -> hs1 (bf16), sub-AG1 per 7 blocks ----
        cc1s = []
        with nc.named_scope("gemm1"):
            for j in range(NSUB):
                wrs = []
                for k in range(BOUND[j], BOUND[j + 1]):
                    kb = slice(k * 128, (k + 1) * 128)
                    lt = lp.tile([128, 2, 128], BF16, tag="lhsT", name="lt")
                    nc.sync.dma_start(out=lt[:],
                                      in_=xT_d[:, kb].rearrange("(i p) c -> p i c", i=2))
                    ps = pzg.tile([128, HID], f32, tag="g1", name="psg")
                    for i in range(2):
                        nc.tensor.matmul(out=ps[:], lhsT=lt[:, i, :], rhs=W1_sb[i][:],
                                         start=(i == 0), stop=(i == 1))
                    nc.scalar.activation(out=hs_own[:, kb], in_=ps[:],
                                         func=mybir.ActivationFunctionType.Copy,
                                         scale=dinv[:, k:k + 1])
                    wrs.append(nc.scalar.dma_start(out=ag1_in[kb, :],
                                                   in_=hs_own[:, kb]))
                cc1s.append(sub_allgather(ag1_in, tab1, BOUND[j], BOUND[j + 1],
                                          wrs, "ag1"))

        def aggregate(k, tab, ccs, self_rhs):
            """Aggregate block k's in-edges (+ self loop) into one 512-wide
            PSUM tile of G1 column groups; returns the psum tile."""
            ps = pp.tile([128, G1 * HID], f32, tag="main", name="psagg")
            # self-loop first: starts the accumulation group (whole 2KB zero
            # region goes pending-zero; untouched cols are zero-filled by the
            # first write that touches them)
            nc.tensor.matmul(out=ps[:, 0:HID], lhsT=ident_bf[:], rhs=self_rhs,
                             start=True, stop=False)
            bc = blk_chunks[k]
            for ci, (coff, ch) in enumerate(bc):
                gt = gp.tile([128, ch + 1, HID], BF16, tag="gt", name="gt")
                gi = nc.gpsimd.dma_gather(
                    gt[:], tab[FBASE:, :], idx_sb[:, coff:coff + _cwidth(ch)],
                    _nidx(ch), kreg[_nidx(ch)], HID, single_packet=False,
                    queue_num=next_q(_nidx(ch)))
                for cc in ccs:
                    add_dep_helper(gi.ins, cc.ins, reason="gather after AG")
                lr = 0
                while lr < ch:
                    m = min(G1, ch - lr)
                    last = (ci == len(bc) - 1) and (lr + m == ch)
                    nc.tensor.matmul(out=ps[:, 0:m * HID], lhsT=ident_bf[:],
                                     rhs=gt[:, lr:lr + m, :], start=False, stop=last)
                    lr += m
            return ps

        def group_sum(ps, tag):
            """Sum the G1 column groups of psum tile ps -> [128, HID] SBUF tile."""
            t = sp_.tile([128, HID], f32, tag=tag, name=tag)
            nc.vector.reduce_sum(out=t[:],
                                 in_=ps[:].rearrange("p (g f) -> p f g", f=HID),
                                 axis=mybir.AxisListType.X)
            return t[:]

        # ---------------- Phase B: layer-1 aggregation -> g (bf16) -------------
        cc2s = []
        with nc.named_scope("agg1"):
            for j in reversed(range(NSUB)):
                wrs = []
                for k in reversed(range(BOUND[j], BOUND[j + 1])):
                    kb = slice(k * 128, (k + 1) * 128)
                    ps = aggregate(k, tab1, cc1s, hs_own[:, kb])
                    s = group_sum(ps, "h1")
                    t1 = sp_.tile([128, HID], f32, tag="t1", name="t1")
                    nc.scalar.activation(out=t1[:], in_=s,
                                         func=mybir.ActivationFunctionType.Copy,
                                         scale=dinv[:, k:k + 1])
                    t2 = sp_.tile([128, HID], f32, tag="t2", name="t2")
                    nc.vector.tensor_add(out=t2[:], in0=t1[:], in1=bias1[:])
                    nc.scalar.activation(out=g_own[:, kb], in_=t2[:],
                                         func=mybir.ActivationFunctionType.Relu,
                                         scale=dinv[:, k:k + 1])
                    wrs.append(nc.scalar.dma_start(out=ag2_in[kb, :], in_=g_own[:, kb]))
                cc2s.append(sub_allgather(ag2_in, tab2, BOUND[j], BOUND[j + 1], wrs, "ag2"))

        # ---------------- Phase C: layer-2 aggregation + GEMM2 -> z (bf16) -----
        cc3s = []
        with nc.named_scope("agg2"):
            for j in reversed(range(NSUB)):
                wrs = []
                for k in reversed(range(BOUND[j], BOUND[j + 1])):
                    kb = slice(k * 128, (k + 1) * 128)
                    ps = aggregate(k, tab2, cc2s, g_own[:, kb])
                    s = group_sum(ps, "h2")
                    qs = sp_.tile([128, HID], BF16, tag="qs", name="qs")
                    nc.scalar.activation(out=qs[:], in_=s,
                                         func=mybir.ActivationFunctionType.Copy,
                                         scale=dinv[:, k:k + 1])
                    qt_ps = pzt.tile([128, 128], BF16, tag="tr", name="trps")
                    nc.tensor.transpose(out=qt_ps[:], in_=qs[:], identity=ident_bf[:])
                    qT = sp_.tile([128, 128], BF16, tag="qT", name="qT")
                    nc.vector.tensor_copy(out=qT[:], in_=qt_ps[:])
                    z_ps = pzy.tile([128, OUT], f32, tag="y", name="zps")
                    nc.tensor.matmul(out=z_ps[:], lhsT=qT[:], rhs=W2_sb[:],
                                     start=True, stop=True)
                    z_bf = sp_.tile([128, 128], BF16, tag="zbf", name="zbf")
                    nc.vector.memset(z_bf[:, OUT:128], 0.0)
                    nc.vector.tensor_add(out=z_bf[:, 0:OUT], in0=z_ps[:], in1=bias2[:])
                    wrs.append(nc.scalar.dma_start(out=ag3_in[kb, :], in_=z_bf[:]))
                cc3s.append(sub_allgather(ag3_in, ztab, BOUND[j], BOUND[j + 1], wrs, "ag3"))

        # ---------------- Phase D: decode ----------------
        # cc3s[-1] is the LAST-FIRED sub-AG (blocks BOUND[0]:BOUND[1]); chunks
        # whose labels all avoid that table group start without waiting for it.
        n_early_rounds = prep["n_early_rounds"]
        n_e2_rounds = prep["n_e2_rounds"]
        with nc.named_scope("decode"):
            out_sb = own.tile([128, ndec_cols], f32)
            col = 0
            for i, (r0, ch) in enumerate(dec_chunks):
                if r0 + ch <= n_e2_rounds:
                    deps = cc3s[:-2]
                elif r0 + ch <= n_early_rounds:
                    deps = cc3s[:-1]
                else:
                    deps = cc3s
                zt = dgp.tile([128, 2 * ch + 1, 128], BF16, tag="za", name="za")
                ga = nc.gpsimd.dma_gather(
                    zt[:], ztab[FBASE:, :], idx_sb[:, dec_offs[i]:dec_offs[i] + _cwidth(2 * ch)],
                    _nidx(2 * ch), kreg[_nidx(2 * ch)], 128, single_packet=False,
                    queue_num=next_q(_nidx(2 * ch)))
                for cc in deps:
                    add_dep_helper(ga.ins, cc.ins, reason="decode after AG3")
                prod = prp.tile([128, ch * OUT], f32, tag="prod", name="prod")
                nc.vector.tensor_mul(out=prod[:].rearrange("p (c o) -> p c o", o=OUT),
                                     in0=zt[:, :ch, 0:OUT], in1=zt[:, ch:2 * ch, 0:OUT])
                nc.vector.reduce_sum(out=out_sb[:, col:col + ch],
                                     in_=prod[:].rearrange("p (c o) -> p c o", o=OUT),
                                     axis=mybir.AxisListType.X)
                nc.sync.dma_start(out=out_d[:, col:col + ch], in_=out_sb[:, col:col + ch])
                col += ch

    lower_extended_insts(nc)
    _fix_sync_waits(nc)
    return nc


def kernel(x, W1, b1, W2, b2, edge_index, edge_label_index):
    x = np.asarray(x, dtype=np.float32)
    W1 = np.asarray(W1, dtype=np.float32)
    b1 = np.asarray(b1, dtype=np.float32)
    W2 = np.asarray(W2, dtype=np.float32)
    b2 = np.asarray(b2, dtype=np.float32)
    prep = _prepare(np.asarray(edge_index), np.asarray(edge_label_index))
    nc = _build(prep)

    xp = np.zeros((NP, IN), dtype=np.float32)
    rm = prep["real_mask"]
    xp[rm] = x[prep["final_perm"][rm]]

    W1_bf = W1.astype(ml_dtypes.bfloat16)
    W2_bf = W2.astype(ml_dtypes.bfloat16)

    in_maps = []
    for c in range(C):
        in_maps.append({
            "xT": np.ascontiguousarray(xp[c * NPC:(c + 1) * NPC].T).astype(ml_dtypes.bfloat16),
            "W1": W1_bf, "b1": b1.reshape(1, HID),
            "W2": W2_bf, "b2": b2.reshape(1, OUT),
            "degp": prep["degp_core"][c],
            "idx16": prep["idx16"][c],
        })
    res = run_bass_kernel_spmd(nc, in_maps, core_ids=list(range(C)))

    LPC = L // C
    out = np.empty(L, dtype=np.float32)
    for c in range(C):
        o = res.results[c]["out"]          # [128, ncols]; slot j at (j%128, j//128)
        j = np.arange(LPC)
        out[c * LPC + prep["label_order"][c]] = o[j % 128, j // 128]
    return out
